# revision 28
# baseline (speedup 1.0000x reference)
"""DeepSeek-style 2-layer MLA transformer forward on 8 Trainium2 NeuronCores.

Sharding: sequence-parallel (data-parallel over tokens) with zigzag query-block
assignment for causal load balance.  Core c owns token blocks (c, NBLK-1-c) of
NBLK=16 blocks of 128 tokens.  Per layer each core computes the kv-lora
projection (wkv_a), its rms norm, the k_pe rope, AND the wkv_b up-projection
(k_nope^T feature-major + V token-major) LOCALLY for its 256 tokens, then two
AllGathers share K ([16*128 k_nope | 64 k_pe | 64 pad] x 256, bf16) and V
(256 x 2048 token-major) -- so the 8.6 GFLOP wkv_b product is computed once
across the fleet instead of 8x replicated.  q / attention / wo / FFN are
computed locally for the core's 256 tokens with replicated weights.
Attention processes the 8 shared key blocks (kb<8, needed by both q-blocks)
at 256-query width and the 8 high blocks at 128 width, halving PE
instruction count vs per-q-block processing.  The head is vocab-parallel:
one AllGather of the final normed x, then each core computes logits
[2048 tokens (slab order), 4000 vocab] from its 16.4MB weight slice (8x less
head-weight DMA), emitted bf16; the host reassembles and upcasts.

Activations are feature-major ([features on partitions, tokens free]) so every
matmul consumes natural-layout weights as the stationary operand.  Attention
scores are computed key-major (S^T[t, s]) so the softmax denominator is a
ones-matmul and P^T needs no transpose for the PV matmul; V is produced
token-major directly by the wkv_b matmul.  SPMD requires one program for all
cores, so the attention loop has a uniform shape (8 key blocks for the low
query block, 16 for the high one) and per-core {0, tri, 1} mask *data* does
the causal selection.  Matmuls are bf16 with fp32 PSUM accumulation; the
residual stream and softmax stats stay fp32.  Norm weights and the score
scale are folded into adjacent weight matrices on the host.
"""

import numpy as np
import ml_dtypes

import concourse.bass as bass
import concourse.mybir as mybir
import concourse.tile as tile
from concourse import bacc
from concourse.bass import IndirectOffsetOnAxis
from concourse.bass_utils import run_bass_kernel_spmd
from concourse.masks import make_identity

F32 = mybir.dt.float32
BF16 = mybir.dt.bfloat16
I32 = mybir.dt.int32
AF = mybir.ActivationFunctionType
ALU = mybir.AluOpType
P = 128
NPBF16 = ml_dtypes.bfloat16


class Cfg:
    def __init__(self, n_cores=8, S=2048, DIM=2048, H=16, KVR=512, INTER=8192,
                 VOCAB=32000, L=2):
        self.n_cores = n_cores
        self.S, self.DIM, self.H, self.KVR = S, DIM, H, KVR
        self.INTER, self.VOCAB, self.L = INTER, VOCAB, L
        self.DN, self.DR, self.DV = 128, 64, 128
        self.QKD = self.DN + self.DR
        self.SCALE = self.QKD ** -0.5
        self.NBLK = S // P
        assert self.NBLK == 2 * n_cores
        self.TLOC = 2 * P
        self.KT = DIM // P
        self.KVT = KVR // P
        self.IT = INTER // P
        self.PE_T = H // 2
        assert H % 4 == 0 and KVR % P == 0 and DIM % 512 == 0
        assert INTER % 512 == 0 and VOCAB % 256 == 0

    def blocks(self, c):
        return (c, self.NBLK - 1 - c)

    def col_of(self, kb):
        """Token-column offset of key block kb in slab (AllGather) order."""
        if kb < self.n_cores:
            return self.TLOC * kb
        return self.TLOC * (self.NBLK - 1 - kb) + P

    def pos_of(self, c):
        b0, b1 = self.blocks(c)
        return np.concatenate([np.arange(P * b0, P * b0 + P),
                               np.arange(P * b1, P * b1 + P)])

    @property
    def n_units0(self):
        return self.n_cores          # key blocks processed for low q-block

    @property
    def n_units1(self):
        return self.NBLK             # key blocks processed for high q-block


# ---------------------------------------------------------------------------
# host-side input preparation
# ---------------------------------------------------------------------------

def _deinterleave_pe(w_pe):
    """[..., 64] interleaved rope dims -> [evens(32) | odds(32)]."""
    return np.concatenate([w_pe[..., 0::2], w_pe[..., 1::2]], axis=-1)


def host_prepare(inputs, c_: Cfg):
    f = lambda a: np.ascontiguousarray(np.asarray(a, np.float32))
    bf = lambda a: np.ascontiguousarray(np.asarray(a).astype(NPBF16))

    tokens = np.asarray(inputs["tokens"]).reshape(-1)
    fc, fs = f(inputs["freqs_cos"]), f(inputs["freqs_sin"])       # [S, 32]

    tri = (np.arange(P)[:, None] <= np.arange(P)[None, :]).astype(np.float32)
    onesb = np.ones((P, P), np.float32)
    zerb = np.zeros((P, P), np.float32)

    shared = {}
    femb = f(inputs["embed"])

    for l in range(c_.L):
        an = f(inputs["attn_norm_w"][l])[:, None]
        wq = f(inputs["wq"][l]) * an * c_.SCALE
        wq = wq.reshape(c_.DIM, c_.H, c_.QKD)
        wq_nope = wq[:, :, :c_.DN].reshape(c_.DIM, c_.H * c_.DN)
        wq_pe = _deinterleave_pe(wq[:, :, c_.DN:]).reshape(c_.DIM,
                                                          c_.H * c_.DR)
        shared[f"wq{l}"] = bf(np.concatenate([wq_nope, wq_pe], 1))

        wkva = f(inputs["wkv_a"][l]) * an
        wkva_pe = _deinterleave_pe(wkva[:, c_.KVR:])
        shared[f"wkva{l}"] = bf(np.concatenate([wkva[:, :c_.KVR], wkva_pe], 1))

        kvn = f(inputs["kv_norm_w"][l])[:, None]
        wkvb = (f(inputs["wkv_b"][l]) * kvn).reshape(c_.KVR, c_.H,
                                                     c_.DN + c_.DV)
        wb_n = wkvb[:, :, :c_.DN].reshape(c_.KVR, c_.H * c_.DN)
        wb_v = wkvb[:, :, c_.DN:].reshape(c_.KVR, c_.H * c_.DV)
        shared[f"wkvb{l}"] = bf(np.concatenate([wb_n, wb_v], 1))

        shared[f"wo{l}"] = bf(inputs["wo"][l])
        fn = f(inputs["ffn_norm_w"][l])[:, None]
        shared[f"w1{l}"] = bf(f(inputs["w1"][l]) * fn)
        shared[f"w3{l}"] = bf(f(inputs["w3"][l]) * fn)
        shared[f"w2{l}"] = bf(inputs["w2"][l])

    headw = bf(f(inputs["head_w"]) * f(inputs["norm_w"])[:, None])
    VS = c_.VOCAB // c_.n_cores

    in_maps = []
    for c in range(c_.n_cores):
        pos = c_.pos_of(c)
        cosT, sinT = fc[pos].T, fs[pos].T                         # [32, 256]
        m = dict(shared)
        m["headw"] = np.ascontiguousarray(headw[:, c * VS:(c + 1) * VS])
        m["gth"] = np.ascontiguousarray(femb[tokens[pos]].T)   # [DIM, 256]
        m["cc_q"] = f(np.concatenate([cosT] * 4, 0))
        m["ss_q"] = f(np.concatenate([-sinT, sinT, -sinT, sinT], 0))
        # block masks: 8 shared masks (qb0 half of shared key blocks kb=j)
        # then 8 extra masks (qb1, load order j = slab -> kb = 15-j).
        qb0, qb1 = c_.blocks(c)
        bm = []
        for j in range(c_.n_cores):
            bm.append(onesb if j < qb0 else (tri if j == qb0 else zerb))
        for j in range(c_.n_cores):
            kb = c_.NBLK - 1 - j
            bm.append(onesb if kb < qb1 else (tri if kb == qb1 else zerb))
        m["bmask"] = bf(np.concatenate(bm, 0))    # [16*128, 128]
        in_maps.append(m)
    return in_maps


# ---------------------------------------------------------------------------
# device program
# ---------------------------------------------------------------------------

def build(nc, c_: Cfg):
    S, DIM, H, KVR, INTER, VOCAB = (c_.S, c_.DIM, c_.H, c_.KVR, c_.INTER,
                                    c_.VOCAB)
    DR = c_.DR
    NU = 2 * c_.n_cores
    d = {}
    d["gth"] = nc.dram_tensor("gth", [DIM, c_.TLOC], F32,
                              kind="ExternalInput")
    d["ccq"] = nc.dram_tensor("cc_q", [P, c_.TLOC], F32, kind="ExternalInput")
    d["ssq"] = nc.dram_tensor("ss_q", [P, c_.TLOC], F32, kind="ExternalInput")
    d["bmask"] = nc.dram_tensor("bmask", [NU * P, P], BF16,
                                kind="ExternalInput")
    for l in range(c_.L):
        d[f"wq{l}"] = nc.dram_tensor(f"wq{l}", [DIM, H * c_.QKD], BF16,
                                     kind="ExternalInput")
        d[f"wkva{l}"] = nc.dram_tensor(f"wkva{l}", [DIM, KVR + DR], BF16,
                                       kind="ExternalInput")
        d[f"wkvb{l}"] = nc.dram_tensor(f"wkvb{l}", [KVR, H * 256], BF16,
                                       kind="ExternalInput")
        d[f"wo{l}"] = nc.dram_tensor(f"wo{l}", [H * c_.DV, DIM], BF16,
                                     kind="ExternalInput")
        d[f"w1{l}"] = nc.dram_tensor(f"w1{l}", [DIM, INTER], BF16,
                                     kind="ExternalInput")
        d[f"w2{l}"] = nc.dram_tensor(f"w2{l}", [INTER, DIM], BF16,
                                     kind="ExternalInput")
        d[f"w3{l}"] = nc.dram_tensor(f"w3{l}", [DIM, INTER], BF16,
                                     kind="ExternalInput")
    VS = VOCAB // c_.n_cores
    d["hw"] = nc.dram_tensor("headw", [DIM, VS], BF16, kind="ExternalInput")
    out_dt = BF16 if getattr(c_, "out_bf16", True) else F32
    d["out"] = nc.dram_tensor("logits", [S, VS], out_dt,
                              kind="ExternalOutput")

    with tile.TileContext(nc) as tc:
        _emit(nc, tc, c_, d)
    nc.compile()


def _emit(nc, tc, c_: Cfg, d):
    S, DIM, H, KVR, INTER, VOCAB = (c_.S, c_.DIM, c_.H, c_.KVR, c_.INTER,
                                    c_.VOCAB)
    KT, KVT, IT, TLOC, NBLK = c_.KT, c_.KVT, c_.IT, c_.TLOC, c_.NBLK
    DR, DV = c_.DR, c_.DV
    NCOR = c_.n_cores
    NU = 2 * c_.n_cores
    KTI = H * c_.DN // P           # 16 k_nope feature tiles
    KROWS = (KTI + 1) * P          # 2176 rows: 16 k_nope tiles + [k_pe|pad]
    VS = c_.VOCAB // NCOR

    import contextlib
    stack = contextlib.ExitStack()
    cpool = stack.enter_context(tc.tile_pool(name="const", bufs=1))
    psum = stack.enter_context(tc.tile_pool(name="psum", bufs=1, space="PSUM"))
    dram = stack.enter_context(tc.tile_pool(name="dram", bufs=1, space="DRAM"))
    hpool = stack.enter_context(tc.tile_pool(name="hres", bufs=1))
    spool = stack.enter_context(tc.tile_pool(name="small", bufs=1))
    _nopex = (None, None, None)

    # ---- constants ----
    ident = cpool.tile([P, P], F32, name="ident")
    make_identity(nc, ident[:])
    ones_bf = cpool.tile([P, 1], BF16, name="ones_bf")
    nc.gpsimd.memset(ones_bf[:], 1.0)
    eps_t = cpool.tile([P, 1], F32, name="eps_t")
    nc.gpsimd.memset(eps_t[:], 1e-6)
    ccq = cpool.tile([P, TLOC], F32, name="ccq_sb")
    nc.sync.dma_start(ccq[:], d["ccq"][:, :])
    ssq = cpool.tile([P, TLOC], F32, name="ssq_sb")
    nc.sync.dma_start(ssq[:], d["ssq"][:, :])
    bm = []
    for u in range(NU):
        t = cpool.tile([P, P], BF16, name=f"bm{u}")
        nc.sync.dma_start(t[:], d["bmask"][u * P:(u + 1) * P, :])
        bm.append(t)

    # ---- residual stream ----
    hT = [hpool.tile([P, TLOC], F32, name=f"hT{i}") for i in range(KT)]

    for rep_i in range(getattr(c_, 'repeat', 1)):
        # ---- embedding rows pre-gathered + transposed on host ----
        gth_r = d["gth"].rearrange("(kt p) c -> p kt c", p=P)
        for k in range(KT):
            eng = nc.sync if k % 2 == 0 else nc.scalar
            eng.dma_start(hT[k][:], gth_r[:, k, :])

        # ---- helpers ----
        def rms(tiles, out_tiles, pool, nm):
            """out = in * rsqrt(mean_over_partition_tiles(in^2) + 1e-6), bf16."""
            nkt = len(tiles)
            W = tiles[0].shape[-1]
            ssq_ps = psum.tile([1, W], F32, name=f"ssqp_{nm}", tag="attv", bufs=2)
            sqs = []
            for i, t in enumerate(tiles):
                sq = pool.tile([P, W], BF16, name=f"sq_{nm}_{i}", tag=f"sq_{nm}",
                               bufs=3)
                nc.vector.tensor_tensor(out=sq[:], in0=t[:], in1=t[:], op=ALU.mult)
                sqs.append(sq)
            acc = pool.tile([P, W], F32, name=f"racc_{nm}", tag="racc",
                            bufs=2)
            nc.vector.tensor_tensor(out=acc[:], in0=sqs[0][:], in1=sqs[1][:],
                                    op=ALU.add)
            for i in range(2, nkt):
                nc.vector.tensor_tensor(out=acc[:], in0=acc[:], in1=sqs[i][:],
                                        op=ALU.add)
            accb = pool.tile([P, W], BF16, name=f"raccb_{nm}", tag="raccb",
                             bufs=2)
            nc.vector.tensor_copy(accb[:], acc[:])
            nc.tensor.matmul(ssq_ps[:1, :], ones_bf[:, :1], accb[:],
                             start=True, stop=True)
            srt = spool.tile([1, W], F32, name=f"srt_{nm}", tag="srt", bufs=2)
            nc.scalar.activation(srt[:1, :], ssq_ps[:1, :], AF.Sqrt,
                                 bias=eps_t[:1, :1], scale=1.0 / (nkt * P))
            rcp = spool.tile([1, W], F32, name=f"rcp_{nm}", tag="rcp", bufs=2)
            nc.vector.reciprocal(rcp[:1, :], srt[:1, :])
            scb = pool.tile([P, W], F32, name=f"scb_{nm}", tag=f"scb_{nm}", bufs=1)
            nc.gpsimd.partition_broadcast(scb[:], rcp[:1, :])
            for i, t in enumerate(tiles):
                nc.vector.tensor_tensor(out=out_tiles[i][:], in0=t[:], in1=scb[:],
                                        op=ALU.mult)

        def rope(ps_ap, rows, cc, ss, outs, pool, nm):
            """ps_ap: [rows, W] fp32 (psum) with [e(32)|o(32)] row groups.
            outs: list of (bf16 out AP [64, W], row0 in ps)."""
            W = ps_ap.shape[-1]
            t1 = pool.tile([P, W], F32, name=f"rt1_{nm}", tag="rt1", bufs=2)
            t2 = pool.tile([P, W], F32, name=f"rt2_{nm}", tag="rt2", bufs=2)
            nc.vector.tensor_tensor(out=t1[:rows, :], in0=ps_ap[:rows, :],
                                    in1=cc[:rows, :], op=ALU.mult)
            for g in range(rows // 32):
                sg = g ^ 1
                nc.vector.tensor_tensor(
                    out=t2[g * 32:(g + 1) * 32, :],
                    in0=ps_ap[sg * 32:(sg + 1) * 32, :],
                    in1=ss[g * 32:(g + 1) * 32, :], op=ALU.mult)
            for out_ap, r0 in outs:
                nc.vector.tensor_tensor(out=out_ap, in0=t1[r0:r0 + 64, :],
                                        in1=t2[r0:r0 + 64, :], op=ALU.add)

        # =======================================================================
        for l in range(c_.L):
            # Pool scoping (LIFO): pA holds q through attention; pD holds the
            # gathered kv (kvg) through attention; pB (xT, local kv_a, q
            # weights) closes after q; pE (v/attention/wo) closes after wo;
            # pF (FFN) is last.
            pA_cm = tc.tile_pool(name=f"qkv{l}", bufs=1)
            pA = pA_cm.__enter__()
            pD_cm = tc.tile_pool(name=f"kvnorm{l}", bufs=1)
            pD = pD_cm.__enter__()
            pB_cm = tc.tile_pool(name=f"x{l}", bufs=1)
            pB = pB_cm.__enter__()
            if True:
                # ---------- rms 1 -> xT ----------
                xTt = pB.tile([P, KT, TLOC], BF16, name=f"xT{l}", tag="xT")
                xT = [xTt[:, i, :] for i in range(KT)]
                rms(hT, xT, pB, f"a{l}")

                # ---------- kv_a LOCAL: kv + rope + rms norm ----------
                wkva_sb = pB.tile([P, KT, KVR + DR], BF16, name=f"wkva{l}",
                                  tag="wkva", bufs=1)
                nc.sync.dma_start(
                    wkva_sb[:, :, :],
                    d[f"wkva{l}"].rearrange("(kt p) c -> p kt c", p=P))
                kvn = pB.tile([P, KVT, TLOC], BF16, name=f"kvn{l}", tag="kvp")
                kpe_loc = pB.tile([64, TLOC], BF16, name=f"kpe{l}", tag="kpel")
                kvft = pB.tile([P, KVT, TLOC], BF16, name=f"kvf{l}", tag="kvf")
                for m in range(KVT):
                    ps = psum.tile([P, TLOC], F32, name=f"kvps{l}_{m}",
                                   tag="mm", bufs=3)
                    for k in range(KT):
                        nc.tensor.matmul(ps[:], wkva_sb[:, k, m * P:(m + 1) * P],
                                         xT[k][:], start=(k == 0),
                                         stop=(k == KT - 1))
                    nc.scalar.copy(kvft[:, m, :], ps[:])
                ps = psum.tile([P, TLOC], F32, name=f"kpps{l}", tag="mm",
                               bufs=3)
                for k in range(KT):
                    nc.tensor.matmul(ps[:64, :], wkva_sb[:, k, KVR:KVR + DR],
                                     xT[k][:], start=(k == 0),
                                     stop=(k == KT - 1))
                rope(ps[:], 64, ccq, ssq, [(kpe_loc[:64, :], 0)],
                     pB, f"k{l}")
                sqt = pB.tile([P, KVT, TLOC], BF16, name=f"kvsq{l}", tag="kvsq")
                for m in range(KVT):
                    nc.vector.tensor_tensor(out=sqt[:, m, :], in0=kvft[:, m, :],
                                            in1=kvft[:, m, :], op=ALU.mult)
                ssq_ps = psum.tile([1, TLOC], F32, name=f"kvssq{l}", tag="attv",
                                   bufs=2)
                for m in range(KVT):
                    nc.tensor.matmul(ssq_ps[:1, :], ones_bf[:, :1],
                                     sqt[:, m, :], start=(m == 0),
                                     stop=(m == KVT - 1))
                srt = spool.tile([1, TLOC], F32, name=f"kvsrt{l}", tag="srt",
                                 bufs=2)
                nc.scalar.activation(srt[:1, :], ssq_ps[:1, :], AF.Sqrt,
                                     bias=eps_t[:1, :1], scale=1.0 / KVR)
                rcp = spool.tile([1, TLOC], F32, name=f"kvrcp{l}", tag="rcp",
                                 bufs=2)
                nc.vector.reciprocal(rcp[:1, :], srt[:1, :])
                scbkv = pB.tile([P, TLOC], F32, name=f"kvscb{l}", tag="kvscb")
                nc.gpsimd.partition_broadcast(scbkv[:], rcp[:1, :])
                for m in range(KVT):
                    nc.vector.tensor_tensor(out=kvn[:, m, :],
                                            in0=kvft[:, m, :], in1=scbkv[:],
                                            op=ALU.mult)

                # ---------- local wkv_b: k_nope^T and V for OWN tokens ------
                # stream wkv_b in 4 chunks of 1024 cols: chunks 0-1 feed the
                # 16 k_nope feature tiles, chunks 2-3 feed V (token-major)
                wkvb_r = d[f"wkvb{l}"].rearrange("(kt p) c -> p kt c", p=P)
                kag = pB.tile([P, KTI + 1, TLOC], BF16, name=f"kag{l}",
                              tag="kag")
                nc.vector.tensor_copy(kag[:64, KTI, :], kpe_loc[:64, :])
                nc.gpsimd.memset(kag[64:, KTI, :], 0.0)
                vag = pB.tile([P, 2, H * DV], BF16, name=f"vag{l}", tag="vag")
                agk_in = dram.tile([KROWS, TLOC], BF16, name=f"agki{l}",
                                   tag="agki", bufs=2)
                agk_out = dram.tile([NCOR * KROWS, TLOC], BF16,
                                    name=f"agko{l}", tag="agko", bufs=2,
                                    addr_space="Shared")
                agv_in = dram.tile([TLOC, H * DV], BF16, name=f"agvi{l}",
                                   tag="agvi", bufs=2)
                agv_out = dram.tile([NCOR * TLOC, H * DV], BF16,
                                    name=f"agvo{l}", tag="agvo", bufs=2,
                                    addr_space="Shared")
                for ch in range(4):
                    wb = pB.tile([P, KVT, 1024], BF16, name=f"wkvb{l}_{ch}",
                                 tag="wkvb", bufs=2)
                    weng = nc.sync if ch % 2 == 0 else nc.scalar
                    weng.dma_start(wb[:, :, :],
                                   wkvb_r[:, :, ch * 1024:ch * 1024 + 1024])
                    if ch < 2:
                        for mi in range(8):
                            m = ch * 8 + mi
                            ps = psum.tile([P, TLOC], F32, name=f"knp{l}_{m}",
                                           tag="mm", bufs=3)
                            for k in range(KVT):
                                nc.tensor.matmul(
                                    ps[:], wb[:, k, mi * P:(mi + 1) * P],
                                    kvn[:, k, :], start=(k == 0),
                                    stop=(k == KVT - 1))
                            nc.any.tensor_copy(kag[:, m, :], ps[:])
                        if ch == 1:
                            # K ready: fire its AllGather before computing V
                            nc.sync.dma_start(
                                agk_in.rearrange("(m p) c -> p m c", p=P),
                                kag[:, :, :])
                            nc.gpsimd.collective_compute(
                                "AllGather", ALU.bypass,
                                replica_groups=[list(range(NCOR))],
                                ins=[agk_in.opt()], outs=[agk_out.opt()])
                    else:
                        for vi in range(2):
                            vc = (ch - 2) * 2 + vi
                            for tt in range(2):
                                ps = psum.tile([P, 512], F32,
                                               name=f"vp{l}_{tt}_{vc}",
                                               tag="mm", bufs=3)
                                for k in range(KVT):
                                    nc.tensor.matmul(
                                        ps[:], kvn[:, k, tt * P:(tt + 1) * P],
                                        wb[:, k, vi * 512:vi * 512 + 512],
                                        start=(k == 0), stop=(k == KVT - 1))
                                nc.any.tensor_copy(
                                    vag[:, tt, vc * 512:vc * 512 + 512], ps[:])

                # ---------- AllGather V (token-major); K already in flight --
                nc.scalar.dma_start(
                    agv_in.rearrange("(t p) c -> p t c", p=P), vag[:, :, :])
                nc.gpsimd.collective_compute(
                    "AllGather", ALU.bypass,
                    replica_groups=[list(range(NCOR))],
                    ins=[agv_in.opt()], outs=[agv_out.opt()])

                # ---------- q projection (local tokens; overlaps AllGather) ----
                qn = [pA.tile([P, TLOC], BF16, name=f"qn{l}_{h}", tag=f"qn{h}")
                      for h in range(H)]
                # qpe padded to 128 partitions (rows 64+ zeroed) so score
                # matmuls keep a uniform 128-row stationary height
                qpe = [pA.tile([P, TLOC], BF16, name=f"qpe{l}_{h}", tag=f"qpe{h}")
                       for h in range(H)]
                for h in range(H):
                    nc.gpsimd.memset(qpe[h][64:, :], 0.0)
                MQ = H + c_.PE_T
                wq_r = d[f"wq{l}"].rearrange("(kt p) c -> p kt c", p=P)
                for mg in range((MQ + 1) // 2):
                    mw = min(256, MQ * P - mg * 256)
                    wqs = pB.tile([P, KT, 256], BF16, name=f"wq{l}_{mg}",
                                  tag="wq", bufs=2)
                    qeng = nc.sync if mg % 2 == 0 else nc.scalar
                    qeng.dma_start(wqs[:, :, :mw],
                                   wq_r[:, :, mg * 256:mg * 256 + mw])
                    for mi in range(2):
                        m = mg * 2 + mi
                        if m >= MQ:
                            break
                        ps = psum.tile([P, TLOC], F32, name=f"qps{l}_{m}",
                                       tag="mm", bufs=3)
                        for k in range(KT):
                            nc.tensor.matmul(ps[:],
                                             wqs[:, k, mi * P:(mi + 1) * P],
                                             xT[k][:], start=(k == 0),
                                             stop=(k == KT - 1))
                        if m < H:
                            nc.any.tensor_copy(qn[m][:], ps[:])
                        else:
                            p = m - H
                            rope(ps[:], P, ccq, ssq,
                                 [(qpe[2 * p][:64, :], 0),
                                  (qpe[2 * p + 1][:64, :], 64)], pB, f"q{l}_{p}")
            pB_cm.__exit__(*_nopex)

            if True:
                # ---------- load gathered V / k_pe (waits on AllGathers) ----
                agk_r = agk_out.rearrange("(r m p) c -> p r m c", r=NCOR,
                                          m=KTI + 1, p=P)
                vsb = [pD.tile([P, H * DV], BF16, name=f"vsb{l}_{t}",
                               tag=f"vsb{t}") for t in range(NBLK)]
                for t in range(NBLK):
                    veng = nc.sync if t % 2 == 0 else nc.scalar
                    veng.dma_start(
                        vsb[t][:, :],
                        agv_out[(t // 2) * TLOC + (t % 2) * P:
                                (t // 2) * TLOC + (t % 2) * P + P, :])
                kpe_sb = pD.tile([P, NCOR, TLOC], BF16, name=f"kpesb{l}",
                                 tag="kpesb")
                nc.sync.dma_start(kpe_sb[:, :, :], agk_r[:, :, KTI, :])

            pV_cm = tc.tile_pool(name=f"vres{l}", bufs=1)
            pV = pV_cm.__enter__()
            pE_cm = tc.tile_pool(name=f"vat{l}", bufs=1)
            pE = pE_cm.__enter__()
            if True:
                # ---------- attention: shared key blocks (kb<8) serve BOTH
                # q-blocks 256-wide; extra blocks (kb>=8, slab order j ->
                # kb=15-j) serve the high q-block 128-wide ----------
                aat = [pV.tile([P, TLOC], BF16, name=f"aat{l}_{i}", tag=f"aat{i}")
                       for i in range(KT)]
                ap = pE
                NSH = c_.n_cores
                for h in range(H):
                    kn_sh = ap.tile([P, NSH, P], BF16, name=f"kns{l}_{h}",
                                    tag="kns", bufs=2)
                    kn_ex = ap.tile([P, NSH, P], BF16, name=f"knx{l}_{h}",
                                    tag="knx", bufs=2)
                    nc.sync.dma_start(kn_sh[:, :, :], agk_r[:, :, h, :P])
                    nc.sync.dma_start(kn_ex[:, :, :], agk_r[:, :, h, P:])
                    att_ps = psum.tile([P, TLOC], F32, name=f"atp{l}_{h}",
                                       tag="attv", bufs=2)
                    den_ps = psum.tile([1, TLOC], F32, name=f"dnp{l}_{h}",
                                       tag="den", bufs=1)
                    es_sh, es_ex = [], []
                    for jp in range(NSH // 2):
                        j0 = 2 * jp
                        sc2 = psum.tile([P, 2, TLOC], F32,
                                        name=f"scp{l}_{h}_{jp}",
                                        tag="sc", bufs=2)
                        for ji in range(2):
                            j = j0 + ji
                            nc.tensor.matmul(sc2[:, ji, :], kn_sh[:, j, :],
                                             qn[h][:, :], start=True,
                                             stop=False,
                                             skip_group_check=True)
                            nc.tensor.matmul(sc2[:, ji, :], kpe_sb[:, j, :P],
                                             qpe[h][:, :], start=False,
                                             stop=True, skip_group_check=True)
                        e2 = ap.tile([P, 2, TLOC], BF16,
                                     name=f"es{l}_{h}_{jp}", tag="es", bufs=6)
                        nc.scalar.activation(e2[:, :, :], sc2[:, :, :], AF.Exp)
                        for ji in range(2):
                            j = j0 + ji
                            nc.vector.tensor_tensor(out=e2[:, ji, :P],
                                                    in0=e2[:, ji, :P],
                                                    in1=bm[j][:], op=ALU.mult)
                            nc.tensor.matmul(
                                att_ps[:], vsb[2 * j][:, h * DV:(h + 1) * DV],
                                e2[:, ji, :], start=(j == 0), stop=False,
                                skip_group_check=True)
                            es_sh.append(e2[:, ji, :])
                    for jp in range(NSH // 2):
                        j0 = 2 * jp
                        sc2 = psum.tile([P, 2, P], F32, name=f"scx{l}_{h}_{jp}",
                                        tag="sc", bufs=2)
                        for ji in range(2):
                            j = j0 + ji
                            nc.tensor.matmul(sc2[:, ji, :], kn_ex[:, j, :],
                                             qn[h][:, P:], start=True,
                                             stop=False,
                                             skip_group_check=True)
                            nc.tensor.matmul(sc2[:, ji, :], kpe_sb[:, j, P:],
                                             qpe[h][:, P:], start=False,
                                             stop=True, skip_group_check=True)
                        e2 = ap.tile([P, 2, P], BF16, name=f"ex{l}_{h}_{jp}",
                                     tag="ex", bufs=6)
                        nc.scalar.activation(e2[:, :, :], sc2[:, :, :], AF.Exp)
                        for ji in range(2):
                            j = j0 + ji
                            nc.vector.tensor_tensor(out=e2[:, ji, :],
                                                    in0=e2[:, ji, :],
                                                    in1=bm[NSH + j][:],
                                                    op=ALU.mult)
                            nc.tensor.matmul(
                                att_ps[:, P:],
                                vsb[2 * j + 1][:, h * DV:(h + 1) * DV],
                                e2[:, ji, :], start=False,
                                stop=(j == NSH - 1),
                                skip_group_check=True)
                            es_ex.append(e2[:, ji, :])
                    # partition-sum via ONE ones-matmul on the DVE-summed
                    # e tiles (16 M=1 matmuls cost ~200ns+ each on PE)
                    dsum = ap.tile([P, TLOC], F32, name=f"ds{l}_{h}",
                                   tag="dsum", bufs=2)
                    nc.vector.tensor_tensor(out=dsum[:], in0=es_sh[0],
                                            in1=es_sh[1], op=ALU.add)
                    for j in range(2, NSH):
                        nc.vector.tensor_tensor(out=dsum[:], in0=dsum[:],
                                                in1=es_sh[j], op=ALU.add)
                    for j in range(NSH):
                        nc.vector.tensor_tensor(out=dsum[:, P:],
                                                in0=dsum[:, P:],
                                                in1=es_ex[j], op=ALU.add)
                    dsb = ap.tile([P, TLOC], BF16, name=f"dsb{l}_{h}",
                                  tag="dsb", bufs=2)
                    nc.vector.tensor_copy(dsb[:], dsum[:])
                    nc.tensor.matmul(den_ps[:1, :], ones_bf[:, :1], dsb[:],
                                     start=True, stop=True)
                    rcp = spool.tile([1, TLOC], F32, name=f"arc{l}_{h}",
                                     tag="rcp", bufs=2)
                    nc.vector.reciprocal(rcp[:1, :], den_ps[:1, :])
                    rb = ap.tile([P, TLOC], F32, name=f"rb{l}_{h}",
                                 tag="rb", bufs=2)
                    nc.gpsimd.partition_broadcast(rb[:], rcp[:1, :])
                    nc.vector.tensor_tensor(out=aat[h][:, :], in0=att_ps[:],
                                            in1=rb[:], op=ALU.mult)

                # ---------- wo + residual ----------
                op = pE
                if True:
                    wo_r = d[f"wo{l}"].rearrange("(kt p) c -> p kt c", p=P)
                    KH = min(8, KT)
                    for mg in range(KT // 4):
                        wos = op.tile([P, KT, 512], BF16, name=f"wo{l}_{mg}",
                                      tag="wo", bufs=2)
                        for kh in range(KT // KH):
                            eng = nc.sync if kh % 2 == 0 else nc.scalar
                            eng.dma_start(
                                wos[:, kh * KH:(kh + 1) * KH, :],
                                wo_r[:, kh * KH:(kh + 1) * KH,
                                     mg * 512:mg * 512 + 512])
                        for mi in range(4):
                            m = mg * 4 + mi
                            ps = psum.tile([P, TLOC], F32, name=f"ops{l}_{m}",
                                           tag="mm", bufs=3)
                            for k in range(KT):
                                nc.tensor.matmul(
                                    ps[:], wos[:, k, mi * P:(mi + 1) * P],
                                    aat[k][:], start=(k == 0), stop=(k == KT - 1))
                            nc.vector.tensor_tensor(out=hT[m][:], in0=hT[m][:],
                                                    in1=ps[:], op=ALU.add)

            pE_cm.__exit__(*_nopex)
            pV_cm.__exit__(*_nopex)
            pD_cm.__exit__(*_nopex)
            pA_cm.__exit__(*_nopex)

            if l == c_.L - 1:
                # head-weight prefetch tiles: allocated outside pF so they
                # survive into the head section; DMAs emitted at the tail of
                # the FFN weight stream (below) so they don't stall it.
                hpre_cm = tc.tile_pool(name="hpre", bufs=1)
                hpre = hpre_cm.__enter__()
                hws_pre = [hpre.tile([P, KT, 500], BF16, name=f"hwpre_{vc}",
                                     tag=f"hwpre{vc}") for vc in range(2)]

            pF_cm = tc.tile_pool(name=f"ffn{l}", bufs=1)
            pF = pF_cm.__enter__()
            if True:
                # ---------- FFN ----------
                fp = pF
                x2T = [pF.tile([P, TLOC], BF16, name=f"x2T{l}_{i}", tag=f"xT{i}_f")
                       for i in range(KT)]
                rms(hT, x2T, pF, f"f{l}")
                gat = [pF.tile([P, TLOC], BF16, name=f"gat{l}_{m}", tag=f"gat{m}")
                       for m in range(IT)]
                w1_r = d[f"w1{l}"].rearrange("(kt p) c -> p kt c", p=P)
                w3_r = d[f"w3{l}"].rearrange("(kt p) c -> p kt c", p=P)
                KH = min(8, KT)
                for mg in range(IT // 4):
                    w1s = fp.tile([P, KT, 512], BF16, name=f"w1{l}_{mg}",
                                  tag="w1", bufs=2)
                    w3s = fp.tile([P, KT, 512], BF16, name=f"w3{l}_{mg}",
                                  tag="w3", bufs=2)
                    for kh in range(KT // KH):
                        ksl = slice(kh * KH, (kh + 1) * KH)
                        nc.sync.dma_start(w1s[:, ksl, :],
                                          w1_r[:, ksl, mg * 512:mg * 512 + 512])
                        nc.scalar.dma_start(
                            w3s[:, ksl, :],
                            w3_r[:, ksl, mg * 512:mg * 512 + 512])
                    for mi in range(4):
                        m = mg * 4 + mi
                        ups = psum.tile([P, TLOC], F32, name=f"ups{l}_{m}",
                                        tag="mm", bufs=3)
                        for k in range(KT):
                            nc.tensor.matmul(
                                ups[:], w1s[:, k, mi * P:(mi + 1) * P],
                                x2T[k][:], start=(k == 0), stop=(k == KT - 1))
                        sg = fp.tile([P, TLOC], BF16, name=f"sg{l}_{m}",
                                     tag="sg", bufs=4)
                        nc.scalar.activation(sg[:], ups[:], AF.Sigmoid)
                        su = fp.tile([P, TLOC], BF16, name=f"su{l}_{m}",
                                     tag="su", bufs=4)
                        nc.vector.tensor_tensor(out=su[:], in0=ups[:], in1=sg[:],
                                                op=ALU.mult)
                        gps = psum.tile([P, TLOC], F32, name=f"gps{l}_{m}",
                                        tag="mm", bufs=3)
                        for k in range(KT):
                            nc.tensor.matmul(
                                gps[:], w3s[:, k, mi * P:(mi + 1) * P],
                                x2T[k][:], start=(k == 0), stop=(k == KT - 1))
                        nc.vector.tensor_tensor(out=gat[m][:], in0=gps[:],
                                                in1=su[:], op=ALU.mult)
                # w2: k-outer accumulation, m-groups of 4 (psum tags borrowed
                # from the idle attention tags to stay within 8 banks)
                w2_r = d[f"w2{l}"].rearrange("(kt p) c -> p kt c", p=P)
                KG = max(1, IT // KH)
                for mg in range(KT // 4):
                    tags = [("mm", 3), ("mm", 3), ("sc", 2), ("attv", 2)]
                    pss = [psum.tile([P, TLOC], F32, name=f"yps{l}_{mg}_{mi}",
                                     tag=tags[mi][0], bufs=tags[mi][1])
                           for mi in range(4)]
                    for kg in range(KG):
                        w2t = fp.tile([P, KH, 512], BF16, name=f"w2{l}_{mg}_{kg}",
                                      tag="w2", bufs=3)
                        eng = nc.sync if kg % 2 == 0 else nc.scalar
                        eng.dma_start(
                            w2t[:, :, :],
                            w2_r[:, kg * KH:(kg + 1) * KH,
                                 mg * 512:mg * 512 + 512])
                        for ki in range(KH):
                            k = kg * KH + ki
                            for mi in range(4):
                                nc.tensor.matmul(
                                    pss[mi][:], w2t[:, ki, mi * P:(mi + 1) * P],
                                    gat[k][:], start=(k == 0),
                                    stop=(k == IT - 1))
                    for mi in range(4):
                        m = mg * 4 + mi
                        nc.vector.tensor_tensor(out=hT[m][:], in0=hT[m][:],
                                                in1=pss[mi][:], op=ALU.add)
                if l == c_.L - 1:
                    hw_pr = d["hw"].rearrange("(kt p) v -> p kt v", p=P)
                    for vc in range(2):
                        for kh in range(2):
                            ksl = slice(kh * 8, (kh + 1) * 8)
                            eng = nc.sync if kh % 2 == 0 else nc.scalar
                            eng.dma_start(
                                hws_pre[vc][:, ksl, :],
                                hw_pr[:, ksl, vc * 500:vc * 500 + 500])
            pF_cm.__exit__(*_nopex)

        # ---------- final norm + AllGather x + vocab-parallel head ----------
        # each core computes logits[ALL tokens (slab order), its VS vocab cols]
        with tc.tile_pool(name="head", bufs=1) as hp:
            xfT = [hp.tile([P, TLOC], BF16, name=f"xfT{i}", tag=f"xfT{i}")
                   for i in range(KT)]
            rms(hT, xfT, hp, "h")
            xg = hp.tile([P, KT, NCOR, TLOC], BF16, name="xg", tag="xg")
            agx_in = dram.tile([DIM, TLOC], BF16, name="agxi", tag="agxi")
            agx_out = dram.tile([NCOR * DIM, TLOC], BF16, name="agxo",
                                tag="agxo", addr_space="Shared")
            for k in range(KT):
                eng = nc.sync if k % 2 == 0 else nc.scalar
                eng.dma_start(
                    agx_in.rearrange("(kt p) c -> p kt c", p=P)[:, k, :],
                    xfT[k][:])
            nc.gpsimd.collective_compute(
                "AllGather", ALU.bypass,
                replica_groups=[list(range(NCOR))],
                ins=[agx_in.opt()], outs=[agx_out.opt()])
            agx_r = agx_out.rearrange("(r kt p) c -> p kt r c", r=NCOR, p=P)
            # slab-major loads: the tt loop touches slabs in order, so the
            # first head matmuls only wait on slab 0's (first) load
            for r in range(NCOR):
                eng = nc.sync if r % 2 == 0 else nc.scalar
                eng.dma_start(xg[:, :, r, :], agx_r[:, :, r, :])
            NV = VS // 500
            hw_r = d["hw"].rearrange("(kt p) v -> p kt v", p=P)
            out_r = d["out"].rearrange("(tt p) v -> p tt v", p=P)
            for vc in range(NV):
                if vc < 2:
                    hws = hws_pre[vc]
                else:
                    hws = hp.tile([P, KT, 500], BF16, name=f"hw_{vc}", tag="hw",
                                  bufs=4)
                    for kh in range(2):
                        ksl = slice(kh * 8, (kh + 1) * 8)
                        eng = nc.sync if kh % 2 == 0 else nc.scalar
                        eng.dma_start(hws[:, ksl, :],
                                      hw_r[:, ksl, vc * 500:vc * 500 + 500])
                for tt in range(NBLK):
                    ps = psum.tile([P, 500], F32, name=f"lps_{vc}_{tt}",
                                   tag="mm", bufs=3)
                    for k in range(KT):
                        nc.tensor.matmul(
                            ps[:], xg[:, k, tt // 2,
                                      (tt % 2) * P:(tt % 2) * P + P],
                            hws[:, k, :], start=(k == 0), stop=(k == KT - 1))
                    lg = hp.tile([P, 500],
                                 BF16 if getattr(c_, "out_bf16", True) else F32,
                                 name=f"lg_{vc}_{tt}", tag="lg", bufs=6)
                    nc.any.tensor_copy(lg[:, :], ps[:, :])
                    nc.scalar.dma_start(
                        out_r[:, tt, vc * 500:vc * 500 + 500], lg[:, :])
        hpre_cm.__exit__(*_nopex)

    stack.close()


# ---------------------------------------------------------------------------
# entry point
# ---------------------------------------------------------------------------

_CACHE = {}


def _get_nc(c_: Cfg):
    key = tuple(sorted(c_.__dict__.items()))
    if key not in _CACHE:
        nc = bacc.Bacc("TRN2", target_bir_lowering=False, debug=False,
                       num_devices=c_.n_cores)
        build(nc, c_)
        _CACHE[key] = nc
    return _CACHE[key]


def kernel(**inputs):
    c_ = Cfg()
    nc = _get_nc(c_)
    in_maps = host_prepare(inputs, c_)
    res = run_bass_kernel_spmd(nc, in_maps,
                               core_ids=list(range(c_.n_cores)))
    out = np.zeros((1, c_.S, c_.VOCAB), np.float32)
    gpos = np.concatenate([c_.pos_of(r) for r in range(c_.n_cores)])
    VS = c_.VOCAB // c_.n_cores
    for c in range(c_.n_cores):
        out[0, gpos, c * VS:(c + 1) * VS] = np.asarray(
            res.results[c]["logits"], np.float32)
    return out

